# revision 14
# baseline (speedup 1.0000x reference)
"""ADMM solver block (nn_ADMMSolverBlock) — Trainium2 Bass kernel, 8 NeuronCores.

Strategy (v2)
-------------
Batch columns sharded 8-per-core (data parallel, no collectives).  The
(I + rho*D^T D)^{-1} solve is separable on the 64x64 grid (D = [I (x) Ax ;
Ay (x) I], verified by runtime probes), handled in the 2D eigenbasis of the
path Laplacians with per-frequency scale S = 1/(1 + rho*(mu_k + nu_l)).

Key algebraic restructure vs v1: the DQ-dependent part of the forward
transform is diagonal in the spectral basis and cancels against S exactly:

    Yh_t = Yh_{t-1} + S .* spectral(rho * D^T U_t),
    U_t  = -(1+eta)*Clip_{t-1} + eta*beta_{t-2}

so only Clip and beta are forward-transformed (2 accumulating matmuls per
half, the beta ones off the critical path).  Spectral state is kept as two
persistent PSUM accumulators  psE  = Vj @ Yh  (Q path) and
psEx = (Ax@Vj) @ Yh (x-gradient path), each fed one matmul per iteration
from the S-scaled increment SK_t.  DQx comes from the psEx lane (transpose
+ Vi), eliminating the gpsimd shifted-difference and any spatial Q until
the final iteration.  C = DQ + beta is accumulated in PSUM (identity
matmul injects eta*beta) so the soft-threshold clamp reads PSUM directly;
Clip is stored pre-scaled by eta, which makes the beta update a single
scalar_tensor_tensor and folds all other scale factors into host-side
constants.  Data tensors and transform constants are fp16 (matmuls run at
1 cycle/row at any free size, transposes 1 cyc/row via an fp16 identity,
and 2-byte operands unlock the DVE 2x/4x modes); accumulation stays fp32
in PSUM, end-to-end relative error ~1e-3.
"""

import numpy as np

import concourse.bacc as bacc
import concourse.mybir as mybir
import concourse.tile as tile
from concourse.bass_utils import run_bass_kernel_spmd

F32 = mybir.dt.float32
F32R = mybir.dt.float32r
F16 = mybir.dt.float16
ALU = mybir.AluOpType

RHO, LAMB, ETA, T = 0.1, 0.01, 0.1, 4
TH = LAMB / RHO
CL = -(1.0 + ETA) / ETA            # Clip-term coefficient on stored eta*Clip
HH = 64
WW = 64
N = HH * WW
BATCH = 64
NCORES = 8
NLB = BATCH // NCORES

C32_NAMES = ["c_vi", "s_t"]                      # f32r block
C16_NAMES = [
    "identb", "c_vj", "c_vjt", "c_axvjt", "c_vite", "c_ayvite",
    "c_vicl", "c_rayvicl", "c_ieta", "c_vib", "c_rayvib", "c_raxvj", "c_vit",
]


def _bd(m):
    out = np.zeros((128, 128), np.float32)
    out[:64, :64] = m
    out[64:, 64:] = m
    return out


def host_constants(D):
    D = np.asarray(D)
    Ax = D[0][:WW, :WW].astype(np.float64)
    Ay = D[1][::WW, ::WW].astype(np.float64)

    rng = np.random.default_rng(0)
    for _ in range(2):
        v = rng.standard_normal(N).astype(np.float32)
        vg = v.reshape(HH, WW)
        if not np.allclose(D[0] @ v, (vg @ Ax.T.astype(np.float32)).ravel(), atol=1e-3):
            raise ValueError("D[0] does not have the expected I (x) Ax structure")
        if not np.allclose(D[1] @ v, (Ay.astype(np.float32) @ vg).ravel(), atol=1e-3):
            raise ValueError("D[1] does not have the expected Ay (x) I structure")

    nu, Vj = np.linalg.eigh(Ax.T @ Ax)
    mu, Vi = np.linalg.eigh(Ay.T @ Ay)
    S = 1.0 / (1.0 + RHO * (mu[:, None] + nu[None, :]))  # S[k, l]

    Vi32 = Vi.astype(np.float32)
    Vj32 = Vj.astype(np.float32)
    ayvi = (Ay @ Vi).astype(np.float32)
    axvj = (Ax @ Vj).astype(np.float32)

    # s_t[(lbr,l), (pair,(c,k))] = S[k, l]  (transposed spectral layout)
    s_t = np.zeros((128, 256), np.float32)
    Sf = S.astype(np.float32)
    for lbr in range(2):
        for pair in range(2):
            for c in range(2):
                s_t[lbr * 64:(lbr + 1) * 64,
                    pair * 128 + c * 64: pair * 128 + (c + 1) * 64] = Sf.T

    return {
        "c_vi": _bd(Vi32),
        "s_t": s_t,
        "identb": np.eye(128, dtype=np.float32),
        "c_vj": _bd(Vj32),
        "c_vjt": _bd(Vj32.T),
        "c_axvjt": _bd(axvj.T),
        "c_vit": _bd(Vi32.T),
        "c_vite": _bd(ETA * Vi32.T),
        "c_ayvite": _bd(ETA * ayvi.T),
        "c_ieta": ETA * np.eye(128, dtype=np.float32),
        "c_vicl": _bd(CL * Vi32),
        "c_vib": _bd(ETA * Vi32),
        "c_rayvicl": _bd(CL * RHO * ayvi),
        "c_rayvib": _bd(ETA * RHO * ayvi),
        "c_raxvj": _bd(RHO * axvj),
    }


def pack_consts(consts):
    import ml_dtypes
    c32 = np.concatenate([consts[n] for n in C32_NAMES], axis=1)
    c16 = np.concatenate([consts[n] for n in C16_NAMES],
                         axis=1).astype(ml_dtypes.float16 if False else np.float16)
    return c32.astype(np.float32), c16


def host_pack(F):
    Fg = np.flip(np.asarray(F), axis=2).transpose(1, 0, 2, 3)  # [c, b, i, j]
    per_core = []
    for r in range(NCORES):
        blk = Fg[:, NLB * r:NLB * (r + 1)]
        per_core.append(np.ascontiguousarray(
            blk.transpose(0, 2, 1, 3).reshape(128, 512)))
    return per_core


def host_unpack(outs):
    Q = np.zeros((BATCH, 2, HH, WW), np.float32)
    for r, o in enumerate(outs):
        blk = o.reshape(2, HH, NLB, WW).transpose(0, 2, 1, 3)  # [c, lb, i, j]
        Q[NLB * r:NLB * (r + 1)] = blk.transpose(1, 0, 2, 3)
    return np.flip(Q, axis=2).copy()


def _mm(nc, out, lhsT, rhs, start=True, stop=True):
    nc.tensor.matmul(out, lhsT=lhsT, rhs=rhs, start=start, stop=stop)


def _tr(nc, out, in_, ident):
    nc.tensor.matmul(out, lhsT=in_, rhs=ident, is_transpose=True,
                     start=True, stop=True)


def build(reps=1, debug=False):
    nc = bacc.Bacc(
        "TRN2",
        target_bir_lowering=False,
        debug=debug,
        enable_asserts=True,
        num_devices=NCORES,
    )
    d_in = nc.dram_tensor("x0", [128, 512], F32R, kind="ExternalInput")
    d_out = nc.dram_tensor("out", [128, 512], F32, kind="ExternalOutput")
    w32 = 128 + 256
    w16 = 128 * len(C16_NAMES)
    d_c32 = nc.dram_tensor("c32", [128, w32], F32R, kind="ExternalInput")
    d_c16 = nc.dram_tensor("c16", [128, w16], F16, kind="ExternalInput")

    with tile.TileContext(nc) as tc:
        with tc.tile_pool(name="const", bufs=1) as cpool, \
             tc.tile_pool(name="state", bufs=1) as spool, \
             tc.tile_pool(name="psum", bufs=1, space="PSUM") as pspool:

            # --- input + const DMAs -----------------------------------
            Fc = spool.tile([128, 512], F32R, tag="Fc", name="Fc")
            for s in range(2):
                nc.sync.dma_start(out=Fc[:, 256 * s:256 * s + 256],
                                  in_=d_in[:, 256 * s:256 * s + 256])
            c32 = cpool.tile([128, w32], F32R, tag="c32", name="c32")
            c16 = cpool.tile([128, w16], F16, tag="c16", name="c16")
            # SWDGE (gpsimd) queue: c_vi, then [identb|c_vj]
            nc.gpsimd.dma_start(out=c32[:, 0:128], in_=d_c32[:, 0:128])
            nc.gpsimd.dma_start(out=c16[:, 0:256], in_=d_c16[:, 0:256])
            # HWDGE (Act) queue: s_t, then mid/late fp16 chunks
            nc.scalar.dma_start(out=c32[:, 128:384], in_=d_c32[:, 128:384])
            nc.scalar.dma_start(out=c16[:, 256:768], in_=d_c16[:, 256:768])
            nc.scalar.dma_start(out=c16[:, 768:w16], in_=d_c16[:, 768:w16])

            ct = {"c_vi": c32[:, 0:128], "s_t": c32[:, 128:384]}
            for idx, name in enumerate(C16_NAMES):
                ct[name] = c16[:, 128 * idx:128 * (idx + 1)]

            # --- PSUM bank map ----------------------------------------
            ps = {}
            for s in range(2):
                ps[f"pAD{s}"] = pspool.tile([128, 512], F32, tag=f"pAD{s}",
                                            name=f"pAD{s}")
                ps[f"pTF{s}"] = pspool.tile([128, 1024], F16, tag=f"pTF{s}",
                                            name=f"pTF{s}")
            ps["pB"] = pspool.tile([128, 512], F32, tag="pB", name="pB")
            for s in range(2):
                ps[f"pEE{s}"] = pspool.tile([128, 512], F32, tag=f"pEE{s}",
                                            name=f"pEE{s}")

            # PE warm-up on the pB bank (finishes before real matmuls).
            wsrc = cpool.tile([128, 128], F32, tag="wsrc", name="wsrc")
            nc.vector.memset(wsrc[:], 1.0)
            for _ in range(8):
                nc.tensor.matmul(ps["pB"][:, 0:128], lhsT=wsrc[:], rhs=wsrc[:],
                                 start=True, stop=True)

            for _ in range(reps):
                _rep(nc, spool, ps, ct, Fc, d_out)

    nc.compile()
    return nc


def _stream_tiles(spool, s):
    return {
        "Asb": spool.tile([128, 512], F16, tag=f"Asb{s}", name=f"Asb{s}"),
        "Tsb": spool.tile([128, 512], F16, tag=f"Tsb{s}", name=f"Tsb{s}"),
        "SK": spool.tile([128, 256], F16, tag=f"SK{s}", name=f"SK{s}"),
        "Esb": spool.tile([128, 256], F16, tag=f"Esb{s}", name=f"Esb{s}"),
        "Exsb": spool.tile([128, 256], F16, tag=f"Exsb{s}", name=f"Exsb{s}"),
        "FF": spool.tile([128, 512], F16, tag=f"FF{s}", name=f"FF{s}"),
        "Clip": spool.tile([128, 512], F16, tag=f"Cl{s}", name=f"Cl{s}"),
        "B0": spool.tile([128, 512], F16, tag=f"B0{s}", name=f"B0{s}"),
        "B1": spool.tile([128, 512], F16, tag=f"B1{s}", name=f"B1{s}"),
        "Yh": spool.tile([128, 256], F16, tag=f"Yh{s}", name=f"Yh{s}"),
        "Qsb": spool.tile([128, 256], F32, tag=f"Q{s}", name=f"Q{s}"),
    }


def _rep(nc, spool, ps, ct, Fc, d_out):
    sts = [_stream_tiles(spool, s) for s in range(2)]
    for t in range(T):
        for s in range(2):
            _iter(nc, ps, ct, sts[s], s, t, Fc, d_out)


def _iter(nc, ps, ct, st, s, t, Fc, d_out):
    idb = ct["identb"]
    pAD = ps[f"pAD{s}"]
    pT = ps[f"pTF{s}"][:, 0:512]
    pFF = ps[f"pTF{s}"][:, 512:1024]
    pB = ps["pB"][:, 256 * s:256 * s + 256]
    pE = ps[f"pEE{s}"][:, 0:256]
    pEx = ps[f"pEE{s}"][:, 256:512]
    last = t == T - 1

    # ---- forward: psA (Vi stage) -> transpose -> psB (Vj stage) -> SK
    if t == 0:
        _mm(nc, pAD[:, 0:256], ct["c_vi"], Fc[:, 256 * s:256 * s + 256])
        nc.vector.tensor_copy(st["Asb"][:, 0:256], pAD[:, 0:256])
        for p in range(2):
            sl = slice(128 * p, 128 * (p + 1))
            _tr(nc, pT[:, sl], st["Asb"][:, sl], idb)
        nc.vector.tensor_copy(st["Tsb"][:, 0:256], pT[:, 0:256])
        _mm(nc, pB, ct["c_vj"], st["Tsb"][:, 0:256])
    else:
        # NOTE: a start=True matmul marks its whole 2KB PSUM zero-region
        # pending — finish each accumulation group before starting the next
        # one in the same bank.
        B2 = st["B0"] if t == 2 else st["B1"]  # beta_{t-2}
        if t >= 2:
            _mm(nc, pAD[:, 0:256], ct["c_vib"], B2[:, 0:256],
                start=True, stop=False)
        _mm(nc, pAD[:, 0:256], ct["c_vicl"], st["Clip"][:, 0:256],
            start=(t == 1), stop=True)
        if t >= 2:
            _mm(nc, pAD[:, 256:512], ct["c_rayvib"], B2[:, 256:512],
                start=True, stop=False)
        _mm(nc, pAD[:, 256:512], ct["c_rayvicl"], st["Clip"][:, 256:512],
            start=(t == 1), stop=True)
        nc.vector.tensor_copy(st["Asb"][:, 0:256], pAD[:, 0:256])
        nc.scalar.copy(st["Asb"][:, 256:512], pAD[:, 256:512])
        for p in range(4):
            sl = slice(128 * p, 128 * (p + 1))
            _tr(nc, pT[:, sl], st["Asb"][:, sl], idb)
        nc.vector.tensor_copy(st["Tsb"][:], pT[:])
        _mm(nc, pB, ct["c_raxvj"], st["Tsb"][:, 0:256], start=True, stop=False)
        _mm(nc, pB, ct["c_vj"], st["Tsb"][:, 256:512], start=False, stop=True)

    nc.vector.tensor_tensor(st["SK"][:], pB, ct["s_t"], ALU.mult)

    # ---- spectral state Yh += SK; fresh psE (Q path) / psEx (x-grad path)
    if t == 0:
        nc.vector.tensor_copy(st["Yh"][:], st["SK"][:])
    else:
        nc.vector.tensor_tensor(st["Yh"][:], st["Yh"][:], st["SK"][:], ALU.add)
    _mm(nc, pE, ct["c_vjt"], st["Yh"][:])
    if not last:
        _mm(nc, pEx, ct["c_axvjt"], st["Yh"][:])

    # ---- inverse: E -> transpose -> spatial gradients / Q
    nc.scalar.copy(st["Esb"][:], pE)
    if not last:
        nc.scalar.copy(st["Exsb"][:], pEx)
    for p in range(2):
        sl = slice(128 * p, 128 * (p + 1))
        _tr(nc, pFF[:, sl], st["Esb"][:, sl], idb)
    if not last:
        for p in range(2):
            sl = slice(128 * p, 128 * (p + 1))
            _tr(nc, pFF[:, 256 + 128 * p:256 + 128 * (p + 1)],
                st["Exsb"][:, sl], idb)
        nc.scalar.copy(st["FF"][:, 0:256], pFF[:, 0:256])
        nc.vector.tensor_copy(st["FF"][:, 256:512], pFF[:, 256:512])
    else:
        nc.vector.tensor_copy(st["FF"][:, 0:256], pFF[:, 0:256])

    if last:
        # final: Q = Vi @ Fsb  -> DRAM
        _mm(nc, pAD[:, 0:256], ct["c_vit"], st["FF"][:, 0:256])
        nc.vector.tensor_copy(st["Qsb"][:], pAD[:, 0:256])
        nc.sync.dma_start(out=d_out[:, 256 * s:256 * s + 256], in_=st["Qsb"][:])
        return

    # ---- psDQP = eta*C = eta*(DQ + beta_{t-1})   (in pAD, reused bank)
    if t >= 1:
        Bp = st["B0"] if t == 1 else st["B1"]  # beta_{t-1}
        _mm(nc, pAD[:, 0:256], ct["c_ieta"], Bp[:, 0:256],
            start=True, stop=False)
    _mm(nc, pAD[:, 0:256], ct["c_vite"], st["FF"][:, 256:512],
        start=(t == 0), stop=True)
    if t >= 1:
        _mm(nc, pAD[:, 256:512], ct["c_ieta"], Bp[:, 256:512],
            start=True, stop=False)
    _mm(nc, pAD[:, 256:512], ct["c_ayvite"], st["FF"][:, 0:256],
        start=(t == 0), stop=True)

    # ---- Clip' = clamp(eta*C, +-eta*th)   (DVE; gpsimd cannot read PSUM)
    nc.vector.tensor_scalar(st["Clip"][:], pAD[:],
                            -ETA * TH, ETA * TH, ALU.max, ALU.min)

    # ---- beta update (only beta_0, beta_1 are ever consumed)
    if t == 0:
        nc.gpsimd.tensor_copy(st["B0"][:], st["Clip"][:])
    elif t == 1:
        nc.vector.scalar_tensor_tensor(st["B1"][:], st["B0"][:], 1.0 - ETA,
                                       st["Clip"][:], ALU.mult, ALU.add)


_CACHE = {}


def _get_nc():
    if "nc" not in _CACHE:
        _CACHE["nc"] = build(reps=1)
    return _CACHE["nc"]


def kernel(F, image, D):
    """Full inputs in, full output out. `image` is unused (mask disabled)."""
    F = np.asarray(F, dtype=np.float32)
    D = np.asarray(D, dtype=np.float32)
    consts = host_constants(D)
    c32, c16 = pack_consts(consts)
    per_core = host_pack(F)
    nc = _get_nc()
    in_maps = [{"x0": per_core[r], "c32": c32, "c16": c16}
               for r in range(NCORES)]
    res = run_bass_kernel_spmd(nc, in_maps, list(range(NCORES)))
    outs = [np.asarray(res.results[r]["out"]) for r in range(NCORES)]
    return host_unpack(outs)


# revision 19
# speedup vs baseline: 1.0363x; 1.0363x over previous
"""ADMM solver block (nn_ADMMSolverBlock) — Trainium2 Bass kernel, 8 NeuronCores.

Strategy (v2)
-------------
Batch columns sharded 8-per-core (data parallel, no collectives).  The
(I + rho*D^T D)^{-1} solve is separable on the 64x64 grid (D = [I (x) Ax ;
Ay (x) I], verified by runtime probes), handled in the 2D eigenbasis of the
path Laplacians with per-frequency scale S = 1/(1 + rho*(mu_k + nu_l)).

Key algebraic restructure vs v1: the DQ-dependent part of the forward
transform is diagonal in the spectral basis and cancels against S exactly:

    Yh_t = Yh_{t-1} + S .* spectral(rho * D^T U_t),
    U_t  = -(1+eta)*Clip_{t-1} + eta*beta_{t-2}

so only Clip and beta are forward-transformed (2 accumulating matmuls per
half, the beta ones off the critical path).  Spectral state is kept as two
persistent PSUM accumulators  psE  = Vj @ Yh  (Q path) and
psEx = (Ax@Vj) @ Yh (x-gradient path), each fed one matmul per iteration
from the S-scaled increment SK_t.  DQx comes from the psEx lane (transpose
+ Vi), eliminating the gpsimd shifted-difference and any spatial Q until
the final iteration.  C = DQ + beta is accumulated in PSUM (identity
matmul injects eta*beta) so the soft-threshold clamp reads PSUM directly;
Clip is stored pre-scaled by eta, which makes the beta update a single
scalar_tensor_tensor and folds all other scale factors into host-side
constants.  Data tensors and transform constants are fp16 (matmuls run at
1 cycle/row at any free size, transposes 1 cyc/row via an fp16 identity,
and 2-byte operands unlock the DVE 2x/4x modes); accumulation stays fp32
in PSUM, end-to-end relative error ~1e-3.
"""

import numpy as np

import concourse.bacc as bacc
import concourse.mybir as mybir
import concourse.tile as tile
from concourse.bass_utils import run_bass_kernel_spmd

F32 = mybir.dt.float32
F32R = mybir.dt.float32r
F16 = mybir.dt.float16
ALU = mybir.AluOpType

RHO, LAMB, ETA, T = 0.1, 0.01, 0.1, 4
TH = LAMB / RHO
CL = -(1.0 + ETA) / ETA            # Clip-term coefficient on stored eta*Clip
HH = 64
WW = 64
N = HH * WW
BATCH = 64
NCORES = 8
NLB = BATCH // NCORES

C32_NAMES = ["c_vi", "s_t"]                      # f32r block
C16_NAMES = [
    "identb", "c_vj", "c_vjt", "c_axvjt", "c_vite", "c_ayvite",
    "c_vicl", "c_rayvicl", "c_ieta", "c_vib", "c_rayvib", "c_raxvj", "c_vit",
]


def _bd(m):
    out = np.zeros((128, 128), np.float32)
    out[:64, :64] = m
    out[64:, 64:] = m
    return out


def host_constants(D):
    D = np.asarray(D)
    Ax = D[0][:WW, :WW].astype(np.float64)
    Ay = D[1][::WW, ::WW].astype(np.float64)

    rng = np.random.default_rng(0)
    for _ in range(2):
        v = rng.standard_normal(N).astype(np.float32)
        vg = v.reshape(HH, WW)
        if not np.allclose(D[0] @ v, (vg @ Ax.T.astype(np.float32)).ravel(), atol=1e-3):
            raise ValueError("D[0] does not have the expected I (x) Ax structure")
        if not np.allclose(D[1] @ v, (Ay.astype(np.float32) @ vg).ravel(), atol=1e-3):
            raise ValueError("D[1] does not have the expected Ay (x) I structure")

    nu, Vj = np.linalg.eigh(Ax.T @ Ax)
    mu, Vi = np.linalg.eigh(Ay.T @ Ay)
    S = 1.0 / (1.0 + RHO * (mu[:, None] + nu[None, :]))  # S[k, l]

    Vi32 = Vi.astype(np.float32)
    Vj32 = Vj.astype(np.float32)
    ayvi = (Ay @ Vi).astype(np.float32)
    axvj = (Ax @ Vj).astype(np.float32)

    # s_t[(lbr,l), (pair,(c,k))] = S[k, l]  (transposed spectral layout)
    s_t = np.zeros((128, 256), np.float32)
    Sf = S.astype(np.float32)
    for lbr in range(2):
        for pair in range(2):
            for c in range(2):
                s_t[lbr * 64:(lbr + 1) * 64,
                    pair * 128 + c * 64: pair * 128 + (c + 1) * 64] = Sf.T

    return {
        "c_vi": _bd(Vi32),
        "s_t": s_t,
        "identb": np.eye(128, dtype=np.float32),
        "c_vj": _bd(Vj32),
        "c_vjt": _bd(Vj32.T),
        "c_axvjt": _bd(axvj.T),
        "c_vit": _bd(Vi32.T),
        "c_vite": _bd(ETA * Vi32.T),
        "c_ayvite": _bd(ETA * ayvi.T),
        "c_ieta": ETA * np.eye(128, dtype=np.float32),
        "c_vicl": _bd(CL * Vi32),
        "c_vib": _bd(ETA * Vi32),
        "c_rayvicl": _bd(CL * RHO * ayvi),
        "c_rayvib": _bd(ETA * RHO * ayvi),
        "c_raxvj": _bd(RHO * axvj),
    }


def pack_consts(consts):
    import ml_dtypes
    c32 = np.concatenate([consts[n] for n in C32_NAMES], axis=1)
    c16 = np.concatenate([consts[n] for n in C16_NAMES],
                         axis=1).astype(ml_dtypes.float16 if False else np.float16)
    return c32.astype(np.float32), c16


def host_pack(F):
    Fg = np.flip(np.asarray(F), axis=2).transpose(1, 0, 2, 3)  # [c, b, i, j]
    per_core = []
    for r in range(NCORES):
        blk = Fg[:, NLB * r:NLB * (r + 1)]
        per_core.append(np.ascontiguousarray(
            blk.transpose(0, 2, 1, 3).reshape(128, 512)))
    return per_core


def host_unpack(outs):
    Q = np.zeros((BATCH, 2, HH, WW), np.float32)
    for r, o in enumerate(outs):
        blk = o.reshape(2, HH, NLB, WW).transpose(0, 2, 1, 3)  # [c, lb, i, j]
        Q[NLB * r:NLB * (r + 1)] = blk.transpose(1, 0, 2, 3)
    return np.flip(Q, axis=2).copy()


def _mm(nc, out, lhsT, rhs, start=True, stop=True):
    nc.tensor.matmul(out, lhsT=lhsT, rhs=rhs, start=start, stop=stop)


def _tr(nc, out, in_, ident):
    nc.tensor.matmul(out, lhsT=in_, rhs=ident, is_transpose=True,
                     start=True, stop=True)


def build(reps=1, debug=False):
    nc = bacc.Bacc(
        "TRN2",
        target_bir_lowering=False,
        debug=debug,
        enable_asserts=True,
        num_devices=NCORES,
    )
    d_in = nc.dram_tensor("x0", [128, 512], F32R, kind="ExternalInput")
    d_out = nc.dram_tensor("out", [128, 512], F32, kind="ExternalOutput")
    w32 = 128 + 256
    w16 = 128 * len(C16_NAMES)
    d_c32 = nc.dram_tensor("c32", [128, w32], F32R, kind="ExternalInput")
    d_c16 = nc.dram_tensor("c16", [128, w16], F16, kind="ExternalInput")

    with tile.TileContext(nc) as tc:
        with tc.tile_pool(name="const", bufs=1) as cpool, \
             tc.tile_pool(name="state", bufs=1) as spool, \
             tc.tile_pool(name="psum", bufs=1, space="PSUM") as pspool:

            # --- input + const DMAs -----------------------------------
            Fc = spool.tile([128, 512], F32R, tag="Fc", name="Fc")
            for s in range(2):
                nc.sync.dma_start(out=Fc[:, 256 * s:256 * s + 256],
                                  in_=d_in[:, 256 * s:256 * s + 256])
            c32 = cpool.tile([128, w32], F32R, tag="c32", name="c32")
            c16 = cpool.tile([128, w16], F16, tag="c16", name="c16")
            # SWDGE (gpsimd) queue: c_vi, then [identb|c_vj]
            nc.gpsimd.dma_start(out=c32[:, 0:128], in_=d_c32[:, 0:128])
            nc.gpsimd.dma_start(out=c16[:, 0:256], in_=d_c16[:, 0:256])
            # HWDGE (Act) queue: s_t, then mid/late fp16 chunks
            nc.scalar.dma_start(out=c32[:, 128:384], in_=d_c32[:, 128:384])
            nc.scalar.dma_start(out=c16[:, 256:768], in_=d_c16[:, 256:768])
            nc.scalar.dma_start(out=c16[:, 768:w16], in_=d_c16[:, 768:w16])

            ct = {"c_vi": c32[:, 0:128], "s_t": c32[:, 128:384]}
            for idx, name in enumerate(C16_NAMES):
                ct[name] = c16[:, 128 * idx:128 * (idx + 1)]

            # --- PSUM bank map ----------------------------------------
            ps = {}
            for s in range(2):
                ps[f"pAD{s}"] = pspool.tile([128, 512], F32, tag=f"pAD{s}",
                                            name=f"pAD{s}")
                ps[f"pTF{s}"] = pspool.tile([128, 1024], F16, tag=f"pTF{s}",
                                            name=f"pTF{s}")
            ps["pB"] = pspool.tile([128, 512], F32, tag="pB", name="pB")
            for s in range(2):
                ps[f"pEE{s}"] = pspool.tile([128, 512], F32, tag=f"pEE{s}",
                                            name=f"pEE{s}")

            # PE warm-up on the pB bank (finishes before real matmuls).
            wsrc = cpool.tile([128, 128], F32, tag="wsrc", name="wsrc")
            nc.vector.memset(wsrc[:], 1.0)
            for _ in range(6):
                nc.tensor.matmul(ps["pB"][:, 0:128], lhsT=wsrc[:], rhs=wsrc[:],
                                 start=True, stop=True)

            for _ in range(reps):
                _rep(nc, spool, ps, ct, Fc, d_out)

    nc.compile()
    return nc


def _stream_tiles(spool, s):
    return {
        "Asb": spool.tile([128, 512], F16, tag=f"Asb{s}", name=f"Asb{s}"),
        "Tsb": spool.tile([128, 512], F16, tag=f"Tsb{s}", name=f"Tsb{s}"),
        "SK": spool.tile([128, 256], F16, tag=f"SK{s}", name=f"SK{s}"),
        "Esb": spool.tile([128, 256], F16, tag=f"Esb{s}", name=f"Esb{s}"),
        "Exsb": spool.tile([128, 256], F16, tag=f"Exsb{s}", name=f"Exsb{s}"),
        "FF": spool.tile([128, 512], F16, tag=f"FF{s}", name=f"FF{s}"),
        "Clip": spool.tile([128, 512], F16, tag=f"Cl{s}", name=f"Cl{s}"),
        "B0": spool.tile([128, 512], F16, tag=f"B0{s}", name=f"B0{s}"),
        "B1": spool.tile([128, 512], F16, tag=f"B1{s}", name=f"B1{s}"),
        "Yh": spool.tile([128, 256], F16, tag=f"Yh{s}", name=f"Yh{s}"),
        "Qsb": spool.tile([128, 256], F32, tag=f"Q{s}", name=f"Q{s}"),
    }


def _rep(nc, spool, ps, ct, Fc, d_out):
    sts = [_stream_tiles(spool, s) for s in range(2)]
    for t in range(T):
        for s in range(2):
            _iter(nc, ps, ct, sts[s], s, t, Fc, d_out)


def _iter(nc, ps, ct, st, s, t, Fc, d_out):
    idb = ct["identb"]
    pAD = ps[f"pAD{s}"]
    pT = ps[f"pTF{s}"][:, 0:512]
    pFF = ps[f"pTF{s}"][:, 512:1024]
    pB = ps["pB"][:, 256 * s:256 * s + 256]
    pE = ps[f"pEE{s}"][:, 0:256]
    pEx = ps[f"pEE{s}"][:, 256:512]
    last = t == T - 1

    # ---- forward: psA (Vi stage) -> transpose -> psB (Vj stage) -> SK
    if t == 0:
        _mm(nc, pAD[:, 0:256], ct["c_vi"], Fc[:, 256 * s:256 * s + 256])
        nc.vector.tensor_copy(st["Asb"][:, 0:256], pAD[:, 0:256])
        for p in range(2):
            sl = slice(128 * p, 128 * (p + 1))
            _tr(nc, pT[:, sl], st["Asb"][:, sl], idb)
        nc.vector.tensor_copy(st["Tsb"][:, 0:256], pT[:, 0:256])
        _mm(nc, pB, ct["c_vj"], st["Tsb"][:, 0:256])
    else:
        # NOTE: a start=True matmul marks its whole 2KB PSUM zero-region
        # pending — finish each accumulation group before starting the next
        # one in the same bank.
        B2 = st["B0"] if t == 2 else st["B1"]  # beta_{t-2}
        if t >= 2:
            _mm(nc, pAD[:, 0:256], ct["c_vib"], B2[:, 0:256],
                start=True, stop=False)
        _mm(nc, pAD[:, 0:256], ct["c_vicl"], st["Clip"][:, 0:256],
            start=(t == 1), stop=True)
        if t >= 2:
            _mm(nc, pAD[:, 256:512], ct["c_rayvib"], B2[:, 256:512],
                start=True, stop=False)
        _mm(nc, pAD[:, 256:512], ct["c_rayvicl"], st["Clip"][:, 256:512],
            start=(t == 1), stop=True)
        nc.vector.tensor_copy(st["Asb"][:, 0:256], pAD[:, 0:256])
        nc.scalar.copy(st["Asb"][:, 256:512], pAD[:, 256:512])
        for p in range(2):
            sl = slice(128 * p, 128 * (p + 1))
            _tr(nc, pT[:, sl], st["Asb"][:, sl], idb)
        nc.vector.tensor_copy(st["Tsb"][:, 0:256], pT[:, 0:256])
        for p in range(2, 4):
            sl = slice(128 * p, 128 * (p + 1))
            _tr(nc, pT[:, sl], st["Asb"][:, sl], idb)
        nc.scalar.copy(st["Tsb"][:, 256:512], pT[:, 256:512])
        _mm(nc, pB, ct["c_raxvj"], st["Tsb"][:, 0:256], start=True, stop=False)
        _mm(nc, pB, ct["c_vj"], st["Tsb"][:, 256:512], start=False, stop=True)

    nc.vector.tensor_tensor(st["SK"][:], pB, ct["s_t"], ALU.mult)

    # ---- spectral state Yh += SK; fresh psE (Q path) / psEx (x-grad path)
    if t == 0:
        nc.vector.tensor_copy(st["Yh"][:], st["SK"][:])
    else:
        nc.vector.tensor_tensor(st["Yh"][:], st["Yh"][:], st["SK"][:], ALU.add)
    _mm(nc, pE, ct["c_vjt"], st["Yh"][:])
    if not last:
        _mm(nc, pEx, ct["c_axvjt"], st["Yh"][:])

    # ---- inverse: E -> transpose -> spatial gradients / Q
    # lane Q (psE -> Esb -> trF -> FF[0:256]) on DVE;
    # lane x (psEx -> Exsb -> trF -> FF[256:512]) on Act, fully parallel.
    nc.vector.tensor_copy(st["Esb"][:], pE)
    if not last:
        nc.scalar.copy(st["Exsb"][:], pEx)
    for p in range(2):
        sl = slice(128 * p, 128 * (p + 1))
        _tr(nc, pFF[:, sl], st["Esb"][:, sl], idb)
    nc.vector.tensor_copy(st["FF"][:, 0:256], pFF[:, 0:256])
    if not last:
        for p in range(2):
            sl = slice(128 * p, 128 * (p + 1))
            _tr(nc, pFF[:, 256 + 128 * p:256 + 128 * (p + 1)],
                st["Exsb"][:, sl], idb)
        nc.scalar.copy(st["FF"][:, 256:512], pFF[:, 256:512])

    if last:
        # final: Q = Vi @ Fsb  -> DRAM
        _mm(nc, pAD[:, 0:256], ct["c_vit"], st["FF"][:, 0:256])
        if s == 0:
            nc.vector.tensor_copy(st["Qsb"][:], pAD[:, 0:256])
        else:
            nc.scalar.copy(st["Qsb"][:], pAD[:, 0:256])
        nc.sync.dma_start(out=d_out[:, 256 * s:256 * s + 256], in_=st["Qsb"][:])
        return

    # ---- psDQP = eta*C = eta*(DQ + beta_{t-1})   (in pAD, reused bank)
    # y half first (its FF-Q input lands first), each half's clamp fires as
    # soon as its own accumulation group stops.
    if t >= 1:
        Bp = st["B0"] if t == 1 else st["B1"]  # beta_{t-1}
        _mm(nc, pAD[:, 256:512], ct["c_ieta"], Bp[:, 256:512],
            start=True, stop=False)
    _mm(nc, pAD[:, 256:512], ct["c_ayvite"], st["FF"][:, 0:256],
        start=(t == 0), stop=True)
    nc.vector.tensor_scalar(st["Clip"][:, 256:512], pAD[:, 256:512],
                            -ETA * TH, ETA * TH, ALU.max, ALU.min)
    if t >= 1:
        _mm(nc, pAD[:, 0:256], ct["c_ieta"], Bp[:, 0:256],
            start=True, stop=False)
    _mm(nc, pAD[:, 0:256], ct["c_vite"], st["FF"][:, 256:512],
        start=(t == 0), stop=True)
    nc.vector.tensor_scalar(st["Clip"][:, 0:256], pAD[:, 0:256],
                            -ETA * TH, ETA * TH, ALU.max, ALU.min)

    # ---- beta update on Pool (only beta_0, beta_1 are ever consumed)
    if t == 0:
        nc.gpsimd.tensor_copy(st["B0"][:], st["Clip"][:])
    elif t == 1:
        nc.gpsimd.tensor_scalar_mul(st["B1"][:], st["B0"][:], 1.0 - ETA)
        nc.gpsimd.tensor_add(st["B1"][:], st["B1"][:], st["Clip"][:])


_CACHE = {}


def _get_nc():
    if "nc" not in _CACHE:
        _CACHE["nc"] = build(reps=1)
    return _CACHE["nc"]


def kernel(F, image, D):
    """Full inputs in, full output out. `image` is unused (mask disabled)."""
    F = np.asarray(F, dtype=np.float32)
    D = np.asarray(D, dtype=np.float32)
    consts = host_constants(D)
    c32, c16 = pack_consts(consts)
    per_core = host_pack(F)
    nc = _get_nc()
    in_maps = [{"x0": per_core[r], "c32": c32, "c16": c16}
               for r in range(NCORES)]
    res = run_bass_kernel_spmd(nc, in_maps, list(range(NCORES)))
    outs = [np.asarray(res.results[r]["out"]) for r in range(NCORES)]
    return host_unpack(outs)
